# revision 26
# baseline (speedup 1.0000x reference)
"""Multi-head attention (B=4, S=2048, D=1024, H=16) on 8 NeuronCores.

Sharding: core c -> (batch b = c//2, head-group g = c%2 of 8 heads).
Per-core: column-parallel fused qkv projection for its 8 heads,
flash-style attention (scores kept transposed: k on partitions so
softmax denominators come from a fused ones-column in the PV matmul),
row-parallel out-projection. The two partial outputs per batch are
summed on the host along with b_out.

HAM note: the PE clock-gates to 1.2 GHz (K=4/8) unless it is the
saturated engine, so the attention mask is applied INSIDE the PE as an
identity-matmul add of -1024*(1-m) (fp8e5m2 mask) accumulated into the
scores PSUM — exp then zeroes masked entries exactly, the DVE mask
multiply disappears, and the PE stays the busiest engine at 2.4 GHz.

Scheduling: one PSUM pool spans all phases (no pool-close drains
between projection / attention / out-projection), and softmax
normalization (reciprocal + broadcast matmuls) is deferred to after
the last head pair so it never stalls the attention pipeline.
"""
import sys

if "/opt/trn_rl_repo" not in sys.path:
    sys.path.insert(0, "/opt/trn_rl_repo")

import numpy as np

B, S, D, H = 4, 2048, 1024, 16
DH = D // H          # 64
HPC = H // 2         # 8 heads per core
CD = HPC * DH        # 512 local head-dims per core
NCORES = 8
VW = HPC * (DH + 1)  # 8 heads x 65 = 520 per s-chunk of vones

_CACHE = {}


def _split_multiwait(nc):
    """walrus in this container accepts ONE sync wait per instruction;
    hoist extras onto injected same-engine EventSemaphore carriers."""
    import concourse.mybir as mybir

    for fn in nc.m.functions:
        for bb in fn.blocks:
            if not any(
                i.sync_info is not None and i.sync_info.on_wait
                and len(i.sync_info.on_wait) > 1
                for i in bb.instructions
            ):
                continue
            newlist = []
            for inst in bb.instructions:
                si = inst.sync_info
                if si is not None and si.on_wait and len(si.on_wait) > 1:
                    waits = list(si.on_wait)
                    for w in waits[:-1]:
                        ev = mybir.InstEventSemaphore(
                            name=nc.get_next_instruction_name(), ins=[], outs=[])
                        ev.engine = inst.engine
                        ev.sync_info = mybir.SyncInfo(on_wait=[w], on_update=[])
                        newlist.append(ev)
                    inst.sync_info = mybir.SyncInfo(
                        on_wait=[waits[-1]], on_update=list(si.on_update))
                newlist.append(inst)
            try:
                bb.instructions = newlist
            except Exception:
                bb.instructions.clear()
                bb.instructions.extend(newlist)


def build_nc(s=S):
    import concourse.bass as bass
    import concourse.mybir as mybir
    from concourse.tile import TileContext

    F32 = mybir.dt.float32
    F32R = mybir.dt.float32r
    BF16 = mybir.dt.bfloat16
    FP8E5 = mybir.dt.float8e5
    EXP = mybir.ActivationFunctionType.Exp
    MULT = mybir.AluOpType.mult

    n_sc = s // 128            # s-chunks of 128
    n_st = s // 512            # s-tiles of 512
    n_kc = s // 128            # k chunks (128 each)
    fd_q = 1024                # q-tile width for attention inner loop
    n_qh = s // fd_q           # q tiles
    n_qn = fd_q // 512         # 512-wide matmuls per q tile

    nc = bass.Bass("TRN2", num_devices=NCORES)

    xT = nc.declare_dram_parameter("xT", [D, s], BF16, isOutput=False)
    wqk = nc.declare_dram_parameter("wqk", [D, 2 * CD], BF16, isOutput=False)
    bqk = nc.declare_dram_parameter("bqk", [128, 8], F32, isOutput=False)
    wv = nc.declare_dram_parameter("wv", [D, CD], BF16, isOutput=False)
    bv = nc.declare_dram_parameter("bv", [1, CD], BF16, isOutput=False)
    mneg = nc.declare_dram_parameter("mneg", [s, s], FP8E5, isOutput=False)
    wout = nc.declare_dram_parameter("wout", [CD, D], BF16, isOutput=False)
    ones_at = nc.declare_dram_parameter("ones_at", [1, 512], BF16, isOutput=False)
    ident = nc.declare_dram_parameter("ident", [128, 128], FP8E5, isOutput=False)
    sel_lo = nc.declare_dram_parameter("sel_lo", [1, 128], F32R, isOutput=False)
    sel_hi = nc.declare_dram_parameter("sel_hi", [1, 128], F32R, isOutput=False)
    y = nc.declare_dram_parameter("y", [s, D], F32, isOutput=True)

    with TileContext(nc) as tc:
        with (
            tc.tile_pool(name="persist", bufs=1) as pp,
            tc.tile_pool(name="poolE", bufs=4) as pe,
            tc.tile_pool(name="poolStg", bufs=2) as pstg,
            tc.tile_pool(name="poolRa", bufs=2) as pra,
            tc.tile_pool(name="poolC", bufs=2) as pc,
            tc.tile_pool(name="ps", bufs=2, space="PSUM") as ps,
        ):
            qkT = pp.tile([128, 8 * s], BF16, tag="qkT")     # [1024 c, s]
            vones = pp.tile([128, n_sc * VW], BF16, tag="vones")
            ctxT = pp.tile([128, 4 * s], BF16, tag="ctxT")   # [512 c, s]
            m01t = pp.tile([128, n_kc * s], FP8E5, tag="m01")
            woutt = pp.tile([128, 4 * D], BF16, tag="wout")
            xt = pp.tile([128, 8 * s], BF16, tag="xt")
            wqkt = pp.tile([128, 8 * 2 * CD], BF16, tag="wqkt")
            wvt = pp.tile([128, 8 * CD], BF16, tag="wvt")
            bqk_t = pp.tile([128, 8], F32, tag="bqk")
            bv_t = pp.tile([1, CD], BF16, tag="bv")
            ones_row = pp.tile([1, 512], BF16, tag="ones")
            ident_t = pp.tile([128, 128], FP8E5, tag="ident")
            sel_lo_t = pp.tile([1, 128], F32R, tag="sel_lo")
            sel_hi_t = pp.tile([1, 128], F32R, tag="sel_hi")

            for dc in range(8):
                nc.sync.dma_start(out=xt[:, dc * s:(dc + 1) * s],
                                  in_=xT[dc * 128:(dc + 1) * 128, :])
                nc.scalar.dma_start(
                    out=wqkt[:, dc * 2 * CD:(dc + 1) * 2 * CD],
                    in_=wqk[dc * 128:(dc + 1) * 128, :])
                nc.gpsimd.dma_start(out=wvt[:, dc * CD:(dc + 1) * CD],
                                    in_=wv[dc * 128:(dc + 1) * 128, :])
            nc.sync.dma_start(out=bqk_t[:], in_=bqk[:])
            nc.sync.dma_start(out=bv_t[:], in_=bv[:])
            nc.sync.dma_start(out=ones_row[:], in_=ones_at[:])
            nc.sync.dma_start(out=ident_t[:], in_=ident[:])
            nc.sync.dma_start(out=sel_lo_t[:], in_=sel_lo[:])
            nc.sync.dma_start(out=sel_hi_t[:], in_=sel_hi[:])
            # ones columns of vones (the rest is overwritten below)
            vones_cols = vones[:].rearrange(
                "p (ch e) -> p ch e", e=DH + 1)[:, :, DH:DH + 1]
            nc.vector.memset(vones_cols, 1.0)
            # mask + wout after the operand loads
            for kc in range(n_kc):
                nc.gpsimd.dma_start(out=m01t[:, kc * s:(kc + 1) * s],
                                    in_=mneg[kc * 128:(kc + 1) * 128, :])
            for ct in range(4):
                nc.gpsimd.dma_start(out=woutt[:, ct * D:(ct + 1) * D],
                                    in_=wout[ct * 128:(ct + 1) * 128, :])

            # ---------------- phase A: qkv projection ----------------
            # psum tags st/ctx are shared across all phases (2 bufs each);
            # phase A keeps two accumulators in flight, one per tag.
            for ct in range(8):
                for sg in range(n_st // 2):
                    pst = [ps.tile([128, 512], F32, tag=t,
                                   name=f"psqk_{ct}_{sg}_{t}")
                           for t in ("st", "ctx")]
                    for dc in range(8):
                        for sti in range(2):
                            st = 2 * sg + sti
                            nc.tensor.matmul(
                                pst[sti][:],
                                lhsT=wqkt[:, dc * 2 * CD + ct * 128:
                                          dc * 2 * CD + (ct + 1) * 128],
                                rhs=xt[:, dc * s + st * 512:
                                       dc * s + (st + 1) * 512],
                                start=(dc == 0), stop=(dc == 7))
                    for sti in range(2):
                        st = 2 * sg + sti
                        nc.vector.tensor_scalar_add(
                            qkT[:, ct * s + st * 512:ct * s + (st + 1) * 512],
                            pst[sti][:],
                            bqk_t[:, ct:ct + 1])

            # v: natural [s, c] layout, bias via a rank-1 matmul
            for scg in range(n_sc // 2):
                psv = [ps.tile([128, 512], F32, tag=t,
                               name=f"psv_{scg}_{t}")
                       for t in ("st", "ctx")]
                for dc in range(8):
                    for sci in range(2):
                        sc = scg * 2 + sci
                        nc.tensor.matmul(
                            psv[sci][:],
                            lhsT=xt[:, dc * s + sc * 128:
                                    dc * s + (sc + 1) * 128],
                            rhs=wvt[:, dc * CD:(dc + 1) * CD],
                            start=(dc == 0), stop=False)
                for sci in range(2):
                    sc = scg * 2 + sci
                    nc.tensor.matmul(
                        psv[sci][:],
                        lhsT=ones_row[0:1, 0:128],
                        rhs=bv_t[0:1, :],
                        start=False, stop=True)
                    dst = vones[:, sc * VW:(sc + 1) * VW].rearrange(
                        "p (h e) -> p h e", e=DH + 1)[:, :, 0:DH]
                    src = psv[sci][:].rearrange("p (h e) -> p h e", e=DH)
                    nc.vector.tensor_copy(dst, src)

            # ---------------- phase B: attention ----------------
            def normalize(hp, qh, rs_q, rcp_q):
                # normalize (hp, qh): ctxT[c, q] *= 1/rowsum (lagged one
                # step so the recip chain never stalls the attention pipe)
                with nc.allow_low_precision(
                        reason="f32r recip feeds f32r broadcast mm"):
                    nc.vector.reciprocal(rcp_q[:], rs_q[:])
                r0a = pra.tile([1, fd_q], F32R, tag="r01",
                               name=f"r0_{hp}_{qh}")
                r1a = pra.tile([1, fd_q], F32R, tag="r01",
                               name=f"r1_{hp}_{qh}")
                nc.sync.dma_start(out=r0a[:], in_=rcp_q[0:1, :])
                nc.sync.dma_start(out=r1a[:], in_=rcp_q[1:2, :])
                bcp = ps.tile([128, fd_q], F32, tag="st",
                              name=f"bcp_{hp}_{qh}")
                for n in range(n_qn):
                    nc.tensor.matmul(
                        bcp[:, n * 512:(n + 1) * 512],
                        lhsT=sel_lo_t[0:1, :],
                        rhs=r0a[0:1, n * 512:(n + 1) * 512],
                        start=True, stop=False)
                    nc.tensor.matmul(
                        bcp[:, n * 512:(n + 1) * 512],
                        lhsT=sel_hi_t[0:1, :],
                        rhs=r1a[0:1, n * 512:(n + 1) * 512],
                        start=False, stop=True)
                sl = ctxT[:, hp * s + qh * fd_q:hp * s + (qh + 1) * fd_q]
                nc.vector.tensor_tensor(sl, sl, bcp[:], MULT)

            pending = None
            for hp in range(4):
                h0, h1 = 2 * hp, 2 * hp + 1
                kt_off = (4 + hp) * s   # K pair c-tile offset in qkT
                qt_off = hp * s         # Q pair c-tile offset
                for qh in range(n_qh):
                    rs_q = pra.tile([2, fd_q], F32R, tag="rsp",
                                    name=f"rs_{hp}_{qh}")
                    rcp_q = pra.tile([2, fd_q], F32R, tag="rcpp",
                                     name=f"rcp_{hp}_{qh}")
                    ctx = [ps.tile([DH + 1, fd_q], F32, tag="ctx",
                                   name=f"ctx_{hp}_{qh}_{i}")
                           for i in range(2)]
                    for kc in range(n_kc):
                        for hi in range(2):
                            pss = ps.tile([128, fd_q], F32, tag="st")
                            r0, r1 = (0, 64) if hi == 0 else (64, 128)
                            for n in range(n_qn):
                                nc.tensor.matmul(
                                    pss[:, n * 512:(n + 1) * 512],
                                    lhsT=qkT[r0:r1,
                                             kt_off + kc * 128:
                                             kt_off + (kc + 1) * 128],
                                    rhs=qkT[r0:r1,
                                            qt_off + qh * fd_q + n * 512:
                                            qt_off + qh * fd_q +
                                            (n + 1) * 512],
                                    start=True, stop=False,
                                    tile_position=(r0, 0))
                            # mask: scores += -1024*(1-m) via identity matmul
                            for n in range(n_qn):
                                nc.tensor.matmul(
                                    pss[:, n * 512:(n + 1) * 512],
                                    lhsT=ident_t[:],
                                    rhs=m01t[:, kc * s + qh * fd_q + n * 512:
                                             kc * s + qh * fd_q +
                                             (n + 1) * 512],
                                    start=False, stop=(n == n_qn - 1),
                                    tile_position=(0, 0))
                            e = pe.tile([128, fd_q], BF16, tag="e")
                            nc.scalar.activation(e[:], pss[:], EXP)
                            h = h0 if hi == 0 else h1
                            for n in range(n_qn):
                                nc.tensor.matmul(
                                    ctx[hi][:, n * 512:(n + 1) * 512],
                                    lhsT=vones[:, kc * VW + h * (DH + 1):
                                               kc * VW + (h + 1) * (DH + 1)],
                                    rhs=e[:, n * 512:(n + 1) * 512],
                                    start=(kc == 0),
                                    stop=(kc == n_kc - 1))
                    # spill unnormalized ctx (DVE) + rowsums (ACT stage ->
                    # DMA into the packed rowsum tile; consumed much later)
                    for hi in range(2):
                        stg = pstg.tile([1, fd_q], F32R, tag="rstg",
                                        name=f"rstg_{hp}_{qh}_{hi}")
                        nc.vector.tensor_copy(stg[:], ctx[hi][DH:DH + 1, :])
                        nc.sync.dma_start(out=rs_q[hi:hi + 1, :],
                                          in_=stg[:])
                        nc.vector.tensor_copy(
                            ctxT[hi * 64:(hi + 1) * 64,
                                 hp * s + qh * fd_q:hp * s + (qh + 1) * fd_q],
                            ctx[hi][0:DH, :])

                    if pending is not None:
                        normalize(*pending)
                    pending = (hp, qh, rs_q, rcp_q)
            normalize(*pending)

            # ---------------- phase C: out projection ----------------
            for qc in range(n_sc):
                for n in range(2):
                    po = ps.tile([128, 512], F32,
                                 tag=("ctx" if (2 * qc + n) % 2 else "st"),
                                 name=f"po_{qc}_{n}")
                    for ct in range(4):
                        nc.tensor.matmul(
                            po[:],
                            lhsT=ctxT[:, ct * s + qc * 128:
                                      ct * s + (qc + 1) * 128],
                            rhs=woutt[:, ct * D + n * 512:
                                      ct * D + (n + 1) * 512],
                            start=(ct == 0), stop=(ct == 3))
                    ot = pc.tile([128, 512], F32, tag="ot")
                    nc.scalar.copy(out=ot[:], in_=po[:])
                    nc.sync.dma_start(
                        out=y[qc * 128:(qc + 1) * 128, n * 512:(n + 1) * 512],
                        in_=ot[:])

    _split_multiwait(nc)
    return nc


def _get_nc(s=S):
    if s not in _CACHE:
        _CACHE[s] = build_nc(s)
    return _CACHE[s]


def make_in_maps(x, W_qkv, b_qkv, W_out, mask, s=S):
    import ml_dtypes
    bf16 = ml_dtypes.bfloat16
    fp8e5 = ml_dtypes.float8_e5m2

    x = np.asarray(x, dtype=np.float32)
    W_qkv = np.asarray(W_qkv, dtype=np.float32)
    b_qkv = np.asarray(b_qkv, dtype=np.float32)
    W_out = np.asarray(W_out, dtype=np.float32)
    mask = np.asarray(mask)
    scale = 1.0 / np.sqrt(DH)
    mneg = np.ascontiguousarray(
        ((mask[0, 0] == 0).T.astype(np.float32) * -1024.0).astype(fp8e5))
    in_maps = []
    for c in range(NCORES):
        b, g = c // 2, c % 2
        wq = W_qkv[:, g * CD:(g + 1) * CD] * scale
        wk = W_qkv[:, D + g * CD:D + (g + 1) * CD]
        bq = b_qkv[g * CD:(g + 1) * CD] * scale
        bk = b_qkv[D + g * CD:D + (g + 1) * CD]
        bqk_col = np.concatenate([bq, bk]).reshape(8, 128).T
        in_maps.append({
            "xT": np.ascontiguousarray(x[b].T.astype(bf16)),
            "wqk": np.ascontiguousarray(
                np.concatenate([wq, wk], axis=1).astype(bf16)),
            "bqk": np.ascontiguousarray(bqk_col.astype(np.float32)),
            "wv": np.ascontiguousarray(
                W_qkv[:, 2 * D + g * CD:2 * D + (g + 1) * CD].astype(bf16)),
            "bv": np.ascontiguousarray(
                b_qkv[2 * D + g * CD:2 * D + (g + 1) * CD][None, :].astype(bf16)),
            "mneg": mneg,
            "wout": np.ascontiguousarray(
                W_out[g * CD:(g + 1) * CD, :].astype(bf16)),
            "ones_at": np.ones((1, 512), dtype=bf16),
            "ident": np.eye(128, dtype=fp8e5),
            "sel_lo": np.concatenate(
                [np.ones(64), np.zeros(64)])[None, :].astype(np.float32),
            "sel_hi": np.concatenate(
                [np.zeros(64), np.ones(64)])[None, :].astype(np.float32),
        })
    return in_maps


def kernel(x, W_qkv, b_qkv, W_out, b_out, mask):
    from concourse.bass_utils import run_bass_kernel_spmd

    nc = _get_nc(S)
    in_maps = make_in_maps(x, W_qkv, b_qkv, W_out, mask, S)
    res = run_bass_kernel_spmd(nc, in_maps, list(range(NCORES)))
    b_out = np.asarray(b_out, dtype=np.float32)
    y = np.empty((B, S, D), dtype=np.float32)
    for b in range(B):
        y[b] = res.results[2 * b]["y"] + res.results[2 * b + 1]["y"] + b_out
    return y


# revision 28
# speedup vs baseline: 1.0230x; 1.0230x over previous
"""Multi-head attention (B=4, S=2048, D=1024, H=16) on 8 NeuronCores.

Sharding: core c -> (batch b = c//2, head-group g = c%2 of 8 heads).
Per-core: column-parallel fused qkv projection for its 8 heads,
flash-style attention (scores kept transposed: k on partitions so
softmax denominators come from a fused ones-column in the PV matmul),
row-parallel out-projection. The two partial outputs per batch are
summed on the host along with b_out.

HAM note: the PE clock-gates to 1.2 GHz (K=4/8) unless it is the
saturated engine, so the attention mask is applied INSIDE the PE as an
identity-matmul add of -1024*(1-m) (fp8e5m2 mask) accumulated into the
scores PSUM — exp then zeroes masked entries exactly, the DVE mask
multiply disappears, and the PE stays the busiest engine at 2.4 GHz.

Scheduling: one PSUM pool spans all phases (no pool-close drains
between projection / attention / out-projection), and softmax
normalization (reciprocal + broadcast matmuls) is deferred to after
the last head pair so it never stalls the attention pipeline.
"""
import sys

if "/opt/trn_rl_repo" not in sys.path:
    sys.path.insert(0, "/opt/trn_rl_repo")

import numpy as np

B, S, D, H = 4, 2048, 1024, 16
DH = D // H          # 64
HPC = H // 2         # 8 heads per core
CD = HPC * DH        # 512 local head-dims per core
NCORES = 8
VW = HPC * (DH + 1)  # 8 heads x 65 = 520 per s-chunk of vones

_CACHE = {}


def _split_multiwait(nc):
    """walrus in this container accepts ONE sync wait per instruction;
    hoist extras onto injected same-engine EventSemaphore carriers."""
    import concourse.mybir as mybir

    for fn in nc.m.functions:
        for bb in fn.blocks:
            if not any(
                i.sync_info is not None and i.sync_info.on_wait
                and len(i.sync_info.on_wait) > 1
                for i in bb.instructions
            ):
                continue
            newlist = []
            for inst in bb.instructions:
                si = inst.sync_info
                if si is not None and si.on_wait and len(si.on_wait) > 1:
                    waits = list(si.on_wait)
                    for w in waits[:-1]:
                        ev = mybir.InstEventSemaphore(
                            name=nc.get_next_instruction_name(), ins=[], outs=[])
                        ev.engine = inst.engine
                        ev.sync_info = mybir.SyncInfo(on_wait=[w], on_update=[])
                        newlist.append(ev)
                    inst.sync_info = mybir.SyncInfo(
                        on_wait=[waits[-1]], on_update=list(si.on_update))
                newlist.append(inst)
            try:
                bb.instructions = newlist
            except Exception:
                bb.instructions.clear()
                bb.instructions.extend(newlist)


def build_nc(s=S):
    import concourse.bass as bass
    import concourse.mybir as mybir
    from concourse.tile import TileContext

    F32 = mybir.dt.float32
    F32R = mybir.dt.float32r
    BF16 = mybir.dt.bfloat16
    FP8E5 = mybir.dt.float8e5
    EXP = mybir.ActivationFunctionType.Exp
    MULT = mybir.AluOpType.mult

    n_sc = s // 128            # s-chunks of 128
    n_st = s // 512            # s-tiles of 512
    n_kc = s // 128            # k chunks (128 each)
    fd_q = 1024                # q-tile width for attention inner loop
    n_qh = s // fd_q           # q tiles
    n_qn = fd_q // 512         # 512-wide matmuls per q tile

    nc = bass.Bass("TRN2", num_devices=NCORES)

    xT = nc.declare_dram_parameter("xT", [D, s], BF16, isOutput=False)
    wqk = nc.declare_dram_parameter("wqk", [D, 2 * CD], BF16, isOutput=False)
    bqk = nc.declare_dram_parameter("bqk", [128, 8], F32, isOutput=False)
    wv = nc.declare_dram_parameter("wv", [D, CD], BF16, isOutput=False)
    bv = nc.declare_dram_parameter("bv", [1, CD], BF16, isOutput=False)
    mneg = nc.declare_dram_parameter("mneg", [s, s], FP8E5, isOutput=False)
    wout = nc.declare_dram_parameter("wout", [CD, D], BF16, isOutput=False)
    ones_at = nc.declare_dram_parameter("ones_at", [1, 512], BF16, isOutput=False)
    ident = nc.declare_dram_parameter("ident", [128, 128], FP8E5, isOutput=False)
    sel_lo = nc.declare_dram_parameter("sel_lo", [1, 128], F32R, isOutput=False)
    sel_hi = nc.declare_dram_parameter("sel_hi", [1, 128], F32R, isOutput=False)
    y = nc.declare_dram_parameter("y", [s, D], F32, isOutput=True)

    with TileContext(nc) as tc:
        with (
            tc.tile_pool(name="persist", bufs=1) as pp,
            tc.tile_pool(name="poolE", bufs=4) as pe,
            tc.tile_pool(name="poolStg", bufs=2) as pstg,
            tc.tile_pool(name="poolRa", bufs=2) as pra,
            tc.tile_pool(name="poolC", bufs=2) as pc,
            tc.tile_pool(name="ps", bufs=2, space="PSUM") as ps,
        ):
            qkT = pp.tile([128, 8 * s], BF16, tag="qkT")     # [1024 c, s]
            vones = pp.tile([128, n_sc * VW], BF16, tag="vones")
            ctxT = pp.tile([128, 4 * s], BF16, tag="ctxT")   # [512 c, s]
            m01t = pp.tile([128, n_kc * s], FP8E5, tag="m01")
            woutt = pp.tile([128, 4 * D], BF16, tag="wout")
            xt = pp.tile([128, 8 * s], BF16, tag="xt")
            wqkt = pp.tile([128, 8 * 2 * CD], BF16, tag="wqkt")
            wvt = pp.tile([128, 8 * CD], BF16, tag="wvt")
            bqk_t = pp.tile([128, 8], F32, tag="bqk")
            bv_t = pp.tile([1, CD], BF16, tag="bv")
            ones_row = pp.tile([1, 512], BF16, tag="ones")
            ident_t = pp.tile([128, 128], FP8E5, tag="ident")
            sel_lo_t = pp.tile([1, 128], F32R, tag="sel_lo")
            sel_hi_t = pp.tile([1, 128], F32R, tag="sel_hi")

            for dc in range(8):
                nc.sync.dma_start(out=xt[:, dc * s:(dc + 1) * s],
                                  in_=xT[dc * 128:(dc + 1) * 128, :])
                nc.scalar.dma_start(
                    out=wqkt[:, dc * 2 * CD:(dc + 1) * 2 * CD],
                    in_=wqk[dc * 128:(dc + 1) * 128, :])
                nc.gpsimd.dma_start(out=wvt[:, dc * CD:(dc + 1) * CD],
                                    in_=wv[dc * 128:(dc + 1) * 128, :])
            nc.sync.dma_start(out=bqk_t[:], in_=bqk[:])
            nc.sync.dma_start(out=bv_t[:], in_=bv[:])
            nc.sync.dma_start(out=ones_row[:], in_=ones_at[:])
            nc.sync.dma_start(out=ident_t[:], in_=ident[:])
            nc.sync.dma_start(out=sel_lo_t[:], in_=sel_lo[:])
            nc.sync.dma_start(out=sel_hi_t[:], in_=sel_hi[:])
            # ones columns of vones (the rest is overwritten below)
            vones_cols = vones[:].rearrange(
                "p (ch e) -> p ch e", e=DH + 1)[:, :, DH:DH + 1]
            nc.vector.memset(vones_cols, 1.0)
            # mask + wout after the operand loads
            for kc in range(n_kc):
                nc.gpsimd.dma_start(out=m01t[:, kc * s:(kc + 1) * s],
                                    in_=mneg[kc * 128:(kc + 1) * 128, :])
            for ct in range(4):
                nc.gpsimd.dma_start(out=woutt[:, ct * D:(ct + 1) * D],
                                    in_=wout[ct * 128:(ct + 1) * 128, :])

            # ---------------- phase A: qkv projection ----------------
            # psum tags st/ctx are shared across all phases (2 bufs each);
            # phase A keeps two accumulators in flight, one per tag.
            for ct in range(8):
                pst = [ps.tile([128, 512], F32, tag=("st", "ctx")[st % 2],
                               name=f"psqk_{ct}_{st}")
                       for st in range(n_st)]
                for dc in range(8):
                    for st in range(n_st):
                        nc.tensor.matmul(
                            pst[st][:],
                            lhsT=wqkt[:, dc * 2 * CD + ct * 128:
                                      dc * 2 * CD + (ct + 1) * 128],
                            rhs=xt[:, dc * s + st * 512:
                                   dc * s + (st + 1) * 512],
                            start=(dc == 0), stop=(dc == 7))
                for st in range(n_st):
                    nc.vector.tensor_scalar_add(
                        qkT[:, ct * s + st * 512:ct * s + (st + 1) * 512],
                        pst[st][:],
                        bqk_t[:, ct:ct + 1])

            # v: natural [s, c] layout, bias via a rank-1 matmul
            for scg in range(n_sc // 2):
                psv = [ps.tile([128, 512], F32, tag=t,
                               name=f"psv_{scg}_{t}")
                       for t in ("st", "ctx")]
                for dc in range(8):
                    for sci in range(2):
                        sc = scg * 2 + sci
                        nc.tensor.matmul(
                            psv[sci][:],
                            lhsT=xt[:, dc * s + sc * 128:
                                    dc * s + (sc + 1) * 128],
                            rhs=wvt[:, dc * CD:(dc + 1) * CD],
                            start=(dc == 0), stop=False)
                for sci in range(2):
                    sc = scg * 2 + sci
                    nc.tensor.matmul(
                        psv[sci][:],
                        lhsT=ones_row[0:1, 0:128],
                        rhs=bv_t[0:1, :],
                        start=False, stop=True)
                    dst = vones[:, sc * VW:(sc + 1) * VW].rearrange(
                        "p (h e) -> p h e", e=DH + 1)[:, :, 0:DH]
                    src = psv[sci][:].rearrange("p (h e) -> p h e", e=DH)
                    nc.vector.tensor_copy(dst, src)

            # ---------------- phase B: attention ----------------
            def normalize(hp, rs_p, rcp_p):
                # normalize pair hp: ctxT[c, q] *= 1/rowsum (emitted one hp
                # late so the recip chain never stalls the attention pipe)
                with nc.allow_low_precision(
                        reason="f32r recip feeds f32r broadcast mm"):
                    nc.vector.reciprocal(rcp_p[:], rs_p[:])
                for qh in range(n_qh):
                    r0a = pra.tile([1, fd_q], F32R, tag="r01",
                                   name=f"r0_{hp}_{qh}")
                    r1a = pra.tile([1, fd_q], F32R, tag="r01",
                                   name=f"r1_{hp}_{qh}")
                    nc.sync.dma_start(out=r0a[:], in_=rcp_p[qh:qh + 1, :])
                    nc.sync.dma_start(out=r1a[:],
                                      in_=rcp_p[2 + qh:2 + qh + 1, :])
                    bcp = ps.tile([128, fd_q], F32, tag="st",
                                  name=f"bcp_{hp}_{qh}")
                    for n in range(n_qn):
                        nc.tensor.matmul(
                            bcp[:, n * 512:(n + 1) * 512],
                            lhsT=sel_lo_t[0:1, :],
                            rhs=r0a[0:1, n * 512:(n + 1) * 512],
                            start=True, stop=False)
                        nc.tensor.matmul(
                            bcp[:, n * 512:(n + 1) * 512],
                            lhsT=sel_hi_t[0:1, :],
                            rhs=r1a[0:1, n * 512:(n + 1) * 512],
                            start=False, stop=True)
                    sl = ctxT[:, hp * s + qh * fd_q:hp * s + (qh + 1) * fd_q]
                    nc.vector.tensor_tensor(sl, sl, bcp[:], MULT)

            pending = None
            for hp in range(4):
                h0, h1 = 2 * hp, 2 * hp + 1
                rs_p = pra.tile([4, fd_q], F32R, tag="rsp",
                                name=f"rs_{hp}")
                rcp_p = pra.tile([4, fd_q], F32R, tag="rcpp",
                                 name=f"rcp_{hp}")
                kt_off = (4 + hp) * s   # K pair c-tile offset in qkT
                qt_off = hp * s         # Q pair c-tile offset
                for qh in range(n_qh):
                    ctx = [ps.tile([DH + 1, fd_q], F32, tag="ctx",
                                   name=f"ctx_{hp}_{qh}_{i}")
                           for i in range(2)]
                    for kc in range(n_kc):
                        for hi in range(2):
                            pss = ps.tile([128, fd_q], F32, tag="st")
                            r0, r1 = (0, 64) if hi == 0 else (64, 128)
                            for n in range(n_qn):
                                nc.tensor.matmul(
                                    pss[:, n * 512:(n + 1) * 512],
                                    lhsT=qkT[r0:r1,
                                             kt_off + kc * 128:
                                             kt_off + (kc + 1) * 128],
                                    rhs=qkT[r0:r1,
                                            qt_off + qh * fd_q + n * 512:
                                            qt_off + qh * fd_q +
                                            (n + 1) * 512],
                                    start=True, stop=False,
                                    tile_position=(r0, 0))
                            # mask: scores += -1024*(1-m) via identity matmul
                            for n in range(n_qn):
                                nc.tensor.matmul(
                                    pss[:, n * 512:(n + 1) * 512],
                                    lhsT=ident_t[:],
                                    rhs=m01t[:, kc * s + qh * fd_q + n * 512:
                                             kc * s + qh * fd_q +
                                             (n + 1) * 512],
                                    start=False, stop=(n == n_qn - 1),
                                    tile_position=(0, 0))
                            e = pe.tile([128, fd_q], BF16, tag="e")
                            nc.scalar.activation(e[:], pss[:], EXP)
                            h = h0 if hi == 0 else h1
                            for n in range(n_qn):
                                nc.tensor.matmul(
                                    ctx[hi][:, n * 512:(n + 1) * 512],
                                    lhsT=vones[:, kc * VW + h * (DH + 1):
                                               kc * VW + (h + 1) * (DH + 1)],
                                    rhs=e[:, n * 512:(n + 1) * 512],
                                    start=(kc == 0),
                                    stop=(kc == n_kc - 1))
                    # spill unnormalized ctx (DVE) + rowsums (ACT stage ->
                    # DMA into the packed rowsum tile; consumed much later)
                    for hi in range(2):
                        stg = pstg.tile([1, fd_q], F32R, tag="rstg",
                                        name=f"rstg_{hp}_{qh}_{hi}")
                        nc.vector.tensor_copy(stg[:], ctx[hi][DH:DH + 1, :])
                        r = 2 * hi + qh
                        nc.sync.dma_start(out=rs_p[r:r + 1, :], in_=stg[:])
                        nc.vector.tensor_copy(
                            ctxT[hi * 64:(hi + 1) * 64,
                                 hp * s + qh * fd_q:hp * s + (qh + 1) * fd_q],
                            ctx[hi][0:DH, :])

                if pending is not None:
                    normalize(*pending)
                pending = (hp, rs_p, rcp_p)
            normalize(*pending)

            # ---------------- phase C: out projection ----------------
            for qc in range(n_sc):
                for n in range(2):
                    po = ps.tile([128, 512], F32,
                                 tag=("ctx" if (2 * qc + n) % 2 else "st"),
                                 name=f"po_{qc}_{n}")
                    for ct in range(4):
                        nc.tensor.matmul(
                            po[:],
                            lhsT=ctxT[:, ct * s + qc * 128:
                                      ct * s + (qc + 1) * 128],
                            rhs=woutt[:, ct * D + n * 512:
                                      ct * D + (n + 1) * 512],
                            start=(ct == 0), stop=(ct == 3))
                    ot = pc.tile([128, 512], F32, tag="ot")
                    nc.scalar.copy(out=ot[:], in_=po[:])
                    nc.sync.dma_start(
                        out=y[qc * 128:(qc + 1) * 128, n * 512:(n + 1) * 512],
                        in_=ot[:])

    _split_multiwait(nc)
    return nc


def _get_nc(s=S):
    if s not in _CACHE:
        _CACHE[s] = build_nc(s)
    return _CACHE[s]


def make_in_maps(x, W_qkv, b_qkv, W_out, mask, s=S):
    import ml_dtypes
    bf16 = ml_dtypes.bfloat16
    fp8e5 = ml_dtypes.float8_e5m2

    x = np.asarray(x, dtype=np.float32)
    W_qkv = np.asarray(W_qkv, dtype=np.float32)
    b_qkv = np.asarray(b_qkv, dtype=np.float32)
    W_out = np.asarray(W_out, dtype=np.float32)
    mask = np.asarray(mask)
    scale = 1.0 / np.sqrt(DH)
    mneg = np.ascontiguousarray(
        ((mask[0, 0] == 0).T.astype(np.float32) * -1024.0).astype(fp8e5))
    in_maps = []
    for c in range(NCORES):
        b, g = c // 2, c % 2
        wq = W_qkv[:, g * CD:(g + 1) * CD] * scale
        wk = W_qkv[:, D + g * CD:D + (g + 1) * CD]
        bq = b_qkv[g * CD:(g + 1) * CD] * scale
        bk = b_qkv[D + g * CD:D + (g + 1) * CD]
        bqk_col = np.concatenate([bq, bk]).reshape(8, 128).T
        in_maps.append({
            "xT": np.ascontiguousarray(x[b].T.astype(bf16)),
            "wqk": np.ascontiguousarray(
                np.concatenate([wq, wk], axis=1).astype(bf16)),
            "bqk": np.ascontiguousarray(bqk_col.astype(np.float32)),
            "wv": np.ascontiguousarray(
                W_qkv[:, 2 * D + g * CD:2 * D + (g + 1) * CD].astype(bf16)),
            "bv": np.ascontiguousarray(
                b_qkv[2 * D + g * CD:2 * D + (g + 1) * CD][None, :].astype(bf16)),
            "mneg": mneg,
            "wout": np.ascontiguousarray(
                W_out[g * CD:(g + 1) * CD, :].astype(bf16)),
            "ones_at": np.ones((1, 512), dtype=bf16),
            "ident": np.eye(128, dtype=fp8e5),
            "sel_lo": np.concatenate(
                [np.ones(64), np.zeros(64)])[None, :].astype(np.float32),
            "sel_hi": np.concatenate(
                [np.zeros(64), np.ones(64)])[None, :].astype(np.float32),
        })
    return in_maps


def kernel(x, W_qkv, b_qkv, W_out, b_out, mask):
    from concourse.bass_utils import run_bass_kernel_spmd

    nc = _get_nc(S)
    in_maps = make_in_maps(x, W_qkv, b_qkv, W_out, mask, S)
    res = run_bass_kernel_spmd(nc, in_maps, list(range(NCORES)))
    b_out = np.asarray(b_out, dtype=np.float32)
    y = np.empty((B, S, D), dtype=np.float32)
    for b in range(B):
        y[b] = res.results[2 * b]["y"] + res.results[2 * b + 1]["y"] + b_out
    return y
